# revision 1
# baseline (speedup 1.0000x reference)
"""Trainium2 Bass kernel for LeNet-C3 binarized 5x5 VALID conv.

out[256,16,124,124] = conv2d(x[256,6,128,128], sign(W)*mask), NCHW/OIHW.

Strategy (per core, data-parallel over batch, 8 cores x 32 images):
  For an output row-block h0..h0+15 the conv is decomposed as 5 PSUM-
  accumulated matmuls (one per kw):
    out[(co,j), (n,w)] += S_kw[(ci,dh), (co,j)]^T @ x[(ci,dh), (n, w+kw)]
  with stationary S_kw[(ci,dh),(co,j)] = wb[co,ci,dh-j,kw] (banded, K=120
  = 6ci x 20dh, M=128 = 8co x 16j).  The kw shift is a free-dim offset into
  the same SBUF tile.  float32r matmul dtype -> 1 cycle/column at N>=256.

  DMA layouts are chosen so every transfer is a fully contiguous 2MB
  block (measured ~2x faster than strided APs on this hw):
    - host pre-packs x into per-h-block [8, 128, npc*128] (rows =
      (ci,dh), padded 120->128; cols = (n,w))
    - kernel writes o as [8, 2, 128, npc*124] ((hb, co-group) blocks,
      rows = (co_l,j), cols = (n,w)); host reassembles.
"""

import sys

sys.path.insert(0, "/opt/trn_rl_repo")

import numpy as np

# ---- problem constants (hardcoded per contract) ----
N_CORES = 8
N, CI, H, WI = 256, 6, 128, 128
CO, KH, KW = 16, 5, 5
HO, WO = 124, 124
NPC = N // N_CORES  # images per core
NSUB = 4  # images per matmul tile (moving N = NSUB*WO = 496 <= 512)
JB = 16  # output rows per block
DH = JB + KH - 1  # input rows per block (20)
KP = CI * DH  # contraction partitions (120)
H0S = [0, 16, 32, 48, 64, 80, 96, 108]  # last block rewrites rows 108..111
NB = len(H0S)
USE_BF16 = False  # bf16 inputs: halves input DMA bytes; weights +-1/0 exact


def _in_dt():
    import concourse.mybir as mybir

    return mybir.dt.bfloat16 if USE_BF16 else mybir.dt.float32r


def _in_np_dt():
    import ml_dtypes

    return ml_dtypes.bfloat16 if USE_BF16 else np.float32

FEATURE_MAPS = [
    [0, 1, 2], [1, 2, 3], [2, 3, 4], [3, 4, 5], [0, 4, 5], [0, 1, 5],
    [0, 1, 2, 3], [1, 2, 3, 4], [2, 3, 4, 5], [0, 3, 4, 5], [0, 1, 4, 5],
    [0, 1, 2, 5], [0, 1, 3, 4], [1, 2, 4, 5], [0, 2, 3, 5],
    [0, 1, 2, 3, 4, 5],
]


def _channel_mask():
    m = np.zeros((CO, CI, 1, 1), np.float32)
    for i, maps in enumerate(FEATURE_MAPS):
        m[i, maps, 0, 0] = 1.0
    return m


def _build_stationary(wb):
    """Banded stationary weights S[g, kw, ci*20+dh, co_l*16+j]."""
    S = np.zeros((2, KW, KP, 128), np.float32)
    for g in range(2):
        for kw in range(KW):
            for col in range(8):
                co = g * 8 + col
                for ci in range(CI):
                    for j in range(JB):
                        for kh in range(KH):
                            S[g, kw, ci * DH + j + kh, col * JB + j] = wb[
                                co, ci, kh, kw
                            ]
    return S


def _pack_x(shard):
    """[npc, CI, H, WI] -> [NB, 128, npc*WI] per-h-block layout."""
    npc = shard.shape[0]
    xt = shard.transpose(1, 2, 0, 3)  # [ci, h, n, w]
    xblk = np.zeros((NB, 128, npc * WI), _in_np_dt())
    for i, h0 in enumerate(H0S):
        xblk[i, :KP] = xt[:, h0 : h0 + DH].reshape(KP, npc * WI).astype(
            _in_np_dt()
        )
    return xblk


def _unpack_o(o_np, npc):
    """[NB, 2, 128, npc*WO] -> [npc, CO, HO, WO]."""
    out = np.empty((npc, CO, HO, WO), np.float32)
    blocks = o_np.reshape(NB, 2, 8, JB, npc, WO)  # hb, g, co_l, j, n, w
    for i, h0 in enumerate(H0S):
        # -> n, g, co_l, j, w
        out[:, :, h0 : h0 + JB, :] = (
            blocks[i].transpose(3, 0, 1, 2, 4).reshape(npc, CO, JB, WO)
        )
    return out


def _body(
    nc,
    x,
    o,
    st,
    xpool,
    opool,
    ppool,
    npc,
    do_load=True,
    do_mm=True,
    do_copy=True,
    do_store=True,
    xfix=None,
    obfix=None,
):
    import concourse.mybir as mybir

    f32 = mybir.dt.float32
    f32r = mybir.dt.float32r
    ngroups = npc // NSUB

    def issue_load(hb):
        xb = xpool.tile([128, npc, WI], _in_dt(), tag="xb")
        leng = nc.sync if hb % 2 == 0 else nc.scalar
        leng.dma_start(xb[:].rearrange("p n w -> p (n w)"), x[hb, :, :])
        return xb

    PREFETCH = 2
    xbs = {}
    if do_load:
        for i in range(min(PREFETCH, NB)):
            xbs[i] = issue_load(i)
    for hb, h0 in enumerate(H0S):
        if do_load:
            # prefetch a later block BEFORE this block's stores hit the rings
            if hb + PREFETCH < NB:
                xbs[hb + PREFETCH] = issue_load(hb + PREFETCH)
            xb = xbs.pop(hb)
        else:
            xb = xfix
        for g in range(2):
            if do_copy:
                ob = opool.tile([128, npc, WO], f32, tag="ob")
            else:
                ob = obfix
            for ng in range(ngroups):
                n0 = ng * NSUB
                if do_mm:
                    ps = ppool.tile([128, NSUB, WO], f32)
                    for kw in range(KW):
                        nc.tensor.matmul(
                            ps[:],
                            st[:, g * KW + kw, :],
                            xb[0:KP, n0 : n0 + NSUB, kw : kw + WO],
                            start=(kw == 0),
                            stop=(kw == KW - 1),
                        )
                    if do_copy:
                        nc.vector.tensor_copy(
                            ob[:, n0 : n0 + NSUB, :], ps[:]
                        )
            if do_store:
                seng = nc.scalar if (hb + g) % 2 == 0 else nc.sync
                seng.dma_start(
                    o[hb, g, :, :], ob[:].rearrange("p n w -> p (n w)")
                )


def build_nc(npc=NPC, reps=1):
    import concourse.mybir as mybir
    import concourse.tile as tile
    from concourse import bacc

    f32 = mybir.dt.float32
    f32r = mybir.dt.float32r

    nc = bacc.Bacc(None, target_bir_lowering=False)
    x = nc.dram_tensor("x", [NB, 128, npc * WI], _in_dt(), kind="ExternalInput")
    s = nc.dram_tensor("s", [2, KW, KP, 128], _in_dt(), kind="ExternalInput")
    o = nc.dram_tensor("o", [NB, 2, 128, npc * WO], f32, kind="ExternalOutput")

    with tile.TileContext(nc) as tc:
        with (
            tc.tile_pool(name="spool", bufs=1) as spool,
            tc.tile_pool(name="xpool", bufs=4) as xpool,
            tc.tile_pool(name="opool", bufs=6) as opool,
            tc.tile_pool(name="ppool", bufs=8, space="PSUM") as ppool,
        ):
            st = spool.tile([KP, 2 * KW, 128], _in_dt())
            nc.sync.dma_start(st[:], s.rearrange("g k p m -> p (g k) m"))
            for _rep in range(reps):
                _body(nc, x, o, st, xpool, opool, ppool, npc)
    nc.compile()
    return nc


def _timing_shell(npc, reps, body_fn, staggered_reset=False, unroll=1, count=True, fixtures=True):
    """Common For_i timing harness: internal DRAM output + rep counter."""
    import concourse.mybir as mybir
    import concourse.tile as tile
    from concourse import bacc

    f32 = mybir.dt.float32
    f32r = mybir.dt.float32r
    ET = mybir.EngineType

    nc = bacc.Bacc(None, target_bir_lowering=False)
    x = nc.dram_tensor("x", [NB, 128, npc * WI], _in_dt(), kind="ExternalInput")
    s = nc.dram_tensor("s", [2, KW, KP, 128], _in_dt(), kind="ExternalInput")
    t = nc.dram_tensor("t", [1, 1], f32, kind="ExternalOutput")

    with tile.TileContext(nc) as tc:
        with (
            tc.tile_pool(name="spool", bufs=1) as spool,
            tc.tile_pool(name="xpool", bufs=4) as xpool,
            tc.tile_pool(name="opool", bufs=6) as opool,
            tc.tile_pool(name="ppool", bufs=8, space="PSUM") as ppool,
            tc.tile_pool(name="dpool", bufs=1, space="DRAM") as dpool,
        ):
            o = dpool.tile([NB, 2, 128, npc * WO], f32)
            st = spool.tile([KP, 2 * KW, 128], _in_dt())
            nc.sync.dma_start(st[:], s.rearrange("g k p m -> p (g k) m"))
            if fixtures:
                xfix = spool.tile([128, npc, WI], _in_dt(), tag="xfix")
                nc.sync.dma_start(
                    xfix[:].rearrange("p n w -> p (n w)"), x[0, :, :]
                )
                obfix = spool.tile([128, npc, WO], f32, tag="obfix")
                nc.gpsimd.memset(obfix[:], 0.25)
            else:
                xfix = obfix = None

            tb = spool.tile([1, 1], f32)
            nc.gpsimd.memset(tb[:], 1.0)
            tzero = spool.tile([1, 1], f32)
            nc.gpsimd.memset(tzero[:], 0.0)
            nc.sync.dma_start(t[:, :], tzero[:])

            def body():
                body_fn(nc, x, o, st, xpool, opool, ppool, xfix, obfix)
                if count:
                    nc.gpsimd.dma_start(
                        t[:, :], tb[:], accum_op=mybir.AluOpType.add
                    )

            if reps == 1:
                body()
            else:
                with tc.For_i(
                    0,
                    (reps - 1) // unroll,
                    1,
                    hint_engines=(ET.PE, ET.Activation, ET.DVE, ET.Pool, ET.SP),
                    staggered_reset=staggered_reset,
                ):
                    body()
                # remainder to make count come out exact
                for _ in range(reps - ((reps - 1) // unroll) * unroll):
                    pass
    nc.compile()
    return nc


def build_nc_timing(reps, npc=NPC):
    def body_fn(nc, x, o, st, xpool, opool, ppool, xfix, obfix):
        _body(nc, x, o, st, xpool, opool, ppool, npc)

    return _timing_shell(npc, reps, body_fn)


def build_nc_micro(which, reps, npc=NPC):
    if which.startswith("u2"):
        which = which[2:]
        unroll = 2
    else:
        unroll = 1
    if which.startswith("sr"):
        which = which[2:]
        stag = True
    else:
        stag = False
    if which.endswith("_nc"):
        which = which[:-3]
        count = False
    else:
        count = True

    flags = {
        "mm": dict(do_load=False, do_copy=False, do_store=False),
        "mmcopy": dict(do_load=False, do_store=False),
        "load": dict(do_mm=False, do_copy=False, do_store=False),
        "store": dict(do_load=False, do_mm=False, do_copy=False),
        "nostore": dict(do_store=False),
        "mcs": dict(do_load=False),
        "lmst": dict(do_copy=False),
        "loadstore": dict(do_mm=False, do_copy=False),
        "full": dict(),
    }[which]

    def body_fn(nc, x, o, st, xpool, opool, ppool, xfix, obfix):
        for _ in range(unroll):
            _body(
                nc, x, o, st, xpool, opool, ppool, npc,
                xfix=xfix, obfix=obfix, **flags,
            )

    return _timing_shell(npc, reps, body_fn, staggered_reset=stag, unroll=unroll, count=count, fixtures=(which != "full"))


_NC_CACHE = {}


def _get_nc(npc=NPC):
    if npc not in _NC_CACHE:
        _NC_CACHE[npc] = build_nc(npc)
    return _NC_CACHE[npc]


def make_in_maps(x, W):
    wb = (np.sign(W) * _channel_mask()).astype(np.float32)
    S = _build_stationary(wb).astype(_in_np_dt())
    shards = x.reshape(N_CORES, NPC, CI, H, WI)
    return [
        {"x": _pack_x(shards[i]), "s": S} for i in range(N_CORES)
    ]


def _run(x, W, trace=False):
    from concourse.bass_utils import run_bass_kernel_spmd

    x = np.asarray(x, dtype=np.float32)
    W = np.asarray(W, dtype=np.float32)
    in_maps = make_in_maps(x, W)
    nc = _get_nc()
    res = run_bass_kernel_spmd(
        nc, in_maps, core_ids=list(range(N_CORES)), trace=trace
    )
    out = np.concatenate(
        [_unpack_o(r["o"], NPC) for r in res.results], axis=0
    )
    return out, res


def kernel(x, W):
    out, _ = _run(x, W, trace=False)
    return out



# revision 4
# speedup vs baseline: 1.7384x; 1.7384x over previous
"""Trainium2 Bass kernel for LeNet-C3 binarized 5x5 VALID conv.

out[256,16,124,124] = conv2d(x[256,6,128,128], sign(W)*mask), NCHW/OIHW.

Strategy (per core, data-parallel over batch, 8 cores x 32 images):
  Parity-split 3-pass matmul scheme, bf16 operands, f32 PSUM accumulation.

  Split w by parity: x[ci, h, 2u+c] -> partition (ci, r, c), free u.
  Per output row-block b (j=4 rows, 31 blocks), input rows 4b..4b+7:
    K = (ci, r in 8, c in 2) = 96 partitions
    M = (co 16, j 4, v 2)   = 128  -> output (co, 4b+j, 2*u0+v)
    3 PSUM-accumulated matmuls (pass p streams free offset u0+p):
      ps[(co,j,v), (n,u0)] += S_p[(ci,r,c),(co,j,v)]^T @ xb[(ci,r,c),(n,u0+p)]
    with S_p[...] = wb[co, ci, r-j, 2p+c-v] (zero where kh/kw out of range).

  vs the 5-pass f32 baseline this is 1.67x fewer PE columns (3*62 vs 5*124
  per 8-image group covering 2x w per column) and ~1.7x less HBM traffic
  (bf16 both directions; input rows 2x-replicated by the 8-row/4-stride
  blocking, outputs written exactly once).

  DMA layouts are partition-major so every transfer is contiguous per
  partition row: x dram [96, 31*2048], o dram [128, 31*1984].
"""

import sys

sys.path.insert(0, "/opt/trn_rl_repo")

import numpy as np

# ---- problem constants (hardcoded per contract) ----
N_CORES = 8
N, CI, H, WI = 256, 6, 128, 128
CO, KH, KW = 16, 5, 5
HO, WO = 124, 124
NPC = N // N_CORES  # images per core (32)

JB = 4              # output rows per block
VB = 2              # output w per psum column (parity pair)
RB = JB + KH - 1    # input rows per block (8)
KP = CI * RB * 2    # contraction partitions (96)
NBLK = HO // JB     # 31 row blocks
NU = WI // 2        # 64 u positions per image
UO = WO // 2        # 62 psum columns per image
NSUB = 8            # images per matmul tile (moving N = 8*62 = 496 <= 1024)
NGRP = NPC // NSUB  # 4 matmul groups per block
NPASS = 3
CB = 4              # row blocks per input DMA chunk (last chunk = 3)
SB = 2              # row blocks per output store

FEATURE_MAPS = [
    [0, 1, 2], [1, 2, 3], [2, 3, 4], [3, 4, 5], [0, 4, 5], [0, 1, 5],
    [0, 1, 2, 3], [1, 2, 3, 4], [2, 3, 4, 5], [0, 3, 4, 5], [0, 1, 4, 5],
    [0, 1, 2, 5], [0, 1, 3, 4], [1, 2, 4, 5], [0, 2, 3, 5],
    [0, 1, 2, 3, 4, 5],
]


def _channel_mask():
    m = np.zeros((CO, CI, 1, 1), np.float32)
    for i, maps in enumerate(FEATURE_MAPS):
        m[i, maps, 0, 0] = 1.0
    return m


def _np_bf16():
    import ml_dtypes

    return ml_dtypes.bfloat16


def _build_stationary(wb):
    """S[p, (ci,r,c), (co,j,v)] = wb[co, ci, r-j, 2p+c-v] where valid."""
    S = np.zeros((NPASS, KP, 128), np.float32)
    for p in range(NPASS):
        for ci in range(CI):
            for r in range(RB):
                for c in range(2):
                    krow = ci * 16 + r * 2 + c
                    for co in range(CO):
                        for j in range(JB):
                            kh = r - j
                            if not (0 <= kh < KH):
                                continue
                            for v in range(VB):
                                kw = 2 * p + c - v
                                if 0 <= kw < KW:
                                    S[p, krow, co * 8 + j * 2 + v] = wb[
                                        co, ci, kh, kw
                                    ]
    return S


def _pack_x(shard):
    """[npc, CI, H, WI] f32 -> [KP, NBLK*npc*NU] bf16 partition-major."""
    npc = shard.shape[0]
    xv = shard.transpose(1, 2, 0, 3)  # [ci, h, n, w]
    xb = np.empty((NBLK, CI, RB, 2, npc, NU), np.float32)
    for b in range(NBLK):
        rows = xv[:, 4 * b: 4 * b + RB]               # [ci, r, n, w]
        xb[b] = rows.reshape(CI, RB, npc, NU, 2).transpose(0, 1, 4, 2, 3)
    # -> partition (ci, r, c), free (b, n, u)
    xp = xb.transpose(1, 2, 3, 0, 4, 5).reshape(KP, NBLK * npc * NU)
    return xp.astype(_np_bf16())


def _unpack_o(o_np, npc):
    """[128, NBLK*npc*UO] bf16 -> [npc, CO, HO, WO] f32."""
    blocks = np.asarray(o_np, dtype=np.float32).reshape(
        CO, JB, VB, NBLK, npc, UO
    )
    # out[n, co, 4b+j, 2u+v]
    out = blocks.transpose(4, 0, 3, 1, 5, 2).reshape(npc, CO, HO, WO)
    return np.ascontiguousarray(out)


def _body(
    nc,
    x,
    o,
    st,
    xpool,
    opool,
    ppool,
    npc,
    do_load=True,
    do_mm=True,
    do_copy=True,
    do_store=True,
    xfix=None,
    obfix=None,
):
    import concourse.mybir as mybir

    f32 = mybir.dt.float32
    bf16 = mybir.dt.bfloat16

    chunk_starts = list(range(0, NBLK, CB))  # [0,4,...,28], last chunk 3 blocks

    def issue_load(cs):
        nblk_c = min(CB, NBLK - cs)
        xt = xpool.tile([KP, CB, npc, NU], bf16, tag="xt")
        nc.sync.dma_start(
            xt[:, 0:nblk_c, :, :].rearrange("p b n u -> p (b n u)"),
            x[:, cs * npc * NU: (cs + nblk_c) * npc * NU],
        )
        return xt

    PREFETCH = 2
    xts = {}
    if do_load:
        for i in range(min(PREFETCH, len(chunk_starts))):
            xts[i] = issue_load(chunk_starts[i])

    ob = None
    for b in range(NBLK):
        ci_, bb = divmod(b, CB)
        if do_load:
            if bb == 0:
                if ci_ + PREFETCH < len(chunk_starts):
                    xts[ci_ + PREFETCH] = issue_load(
                        chunk_starts[ci_ + PREFETCH]
                    )
                if ci_ - 1 in xts:
                    del xts[ci_ - 1]
            xt = xts[ci_]
        else:
            xt = xfix
            bb = 0
        sb = b % SB
        if do_copy and sb == 0:
            ob = opool.tile([128, SB, npc, UO], bf16, tag="ob")
        elif not do_copy:
            ob = obfix
        # pairwise-interleaved matmul groups to space PSUM deps
        for ng0 in range(0, NGRP, 2):
            if do_mm:
                n0a, n0b = ng0 * NSUB, (ng0 + 1) * NSUB
                psa = ppool.tile([128, NSUB, UO], f32, tag="ps")
                psb = ppool.tile([128, NSUB, UO], f32, tag="ps")
                for p in range(NPASS):
                    nc.tensor.matmul(
                        psa[:],
                        st[:, p, :],
                        xt[:, bb, n0a: n0a + NSUB, p: p + UO],
                        start=(p == 0),
                        stop=(p == NPASS - 1),
                    )
                    nc.tensor.matmul(
                        psb[:],
                        st[:, p, :],
                        xt[:, bb, n0b: n0b + NSUB, p: p + UO],
                        start=(p == 0),
                        stop=(p == NPASS - 1),
                    )
                if do_copy:
                    nc.vector.tensor_copy(
                        ob[:, sb, n0a: n0a + NSUB, :], psa[:]
                    )
                    nc.vector.tensor_copy(
                        ob[:, sb, n0b: n0b + NSUB, :], psb[:]
                    )
        if do_store and (sb == SB - 1 or b == NBLK - 1):
            nsb = sb + 1
            b0 = b - sb
            nc.scalar.dma_start(
                o[:, b0 * npc * UO: (b0 + nsb) * npc * UO],
                ob[:, 0:nsb, :, :].rearrange("p b n u -> p (b n u)"),
            )


def build_nc(npc=NPC, reps=1):
    import concourse.mybir as mybir
    import concourse.tile as tile
    from concourse import bacc

    bf16 = mybir.dt.bfloat16

    nc = bacc.Bacc(None, target_bir_lowering=False)
    x = nc.dram_tensor(
        "x", [KP, NBLK * npc * NU], bf16, kind="ExternalInput"
    )
    s = nc.dram_tensor("s", [KP, NPASS, 128], bf16, kind="ExternalInput")
    o = nc.dram_tensor(
        "o", [128, NBLK * npc * UO], bf16, kind="ExternalOutput"
    )

    with tile.TileContext(nc) as tc:
        with (
            tc.tile_pool(name="spool", bufs=1) as spool,
            tc.tile_pool(name="xpool", bufs=3) as xpool,
            tc.tile_pool(name="opool", bufs=4) as opool,
            tc.tile_pool(name="ppool", bufs=8, space="PSUM") as ppool,
        ):
            st = spool.tile([KP, NPASS, 128], bf16)
            nc.sync.dma_start(st[:], s[:, :, :].rearrange("p a m -> p (a m)"))
            for _rep in range(reps):
                _body(nc, x, o, st, xpool, opool, ppool, npc)
    nc.compile()
    return nc


def _timing_shell(
    npc, reps, body_fn, staggered_reset=False, unroll=1, count=True,
    fixtures=True,
):
    """Common For_i timing harness: internal DRAM output + rep counter."""
    import concourse.mybir as mybir
    import concourse.tile as tile
    from concourse import bacc

    f32 = mybir.dt.float32
    bf16 = mybir.dt.bfloat16
    ET = mybir.EngineType

    nc = bacc.Bacc(None, target_bir_lowering=False)
    x = nc.dram_tensor(
        "x", [KP, NBLK * npc * NU], bf16, kind="ExternalInput"
    )
    s = nc.dram_tensor("s", [KP, NPASS, 128], bf16, kind="ExternalInput")
    t = nc.dram_tensor("t", [1, 1], f32, kind="ExternalOutput")

    with tile.TileContext(nc) as tc:
        with (
            tc.tile_pool(name="spool", bufs=1) as spool,
            tc.tile_pool(name="xpool", bufs=3) as xpool,
            tc.tile_pool(name="opool", bufs=4) as opool,
            tc.tile_pool(name="ppool", bufs=8, space="PSUM") as ppool,
            tc.tile_pool(name="dpool", bufs=1, space="DRAM") as dpool,
        ):
            o = dpool.tile([128, NBLK * npc * UO], bf16)
            st = spool.tile([KP, NPASS, 128], bf16)
            nc.sync.dma_start(st[:], s[:, :, :].rearrange("p a m -> p (a m)"))
            if fixtures:
                xfix = spool.tile([KP, 1, npc, NU], bf16, tag="xfix")
                nc.sync.dma_start(
                    xfix[:].rearrange("p b n u -> p (b n u)"),
                    x[:, 0: npc * NU],
                )
                obfix = spool.tile([128, SB, npc, UO], bf16, tag="obfix")
                nc.gpsimd.memset(obfix[:], 0.25)
            else:
                xfix = obfix = None

            tb = spool.tile([1, 1], f32)
            nc.gpsimd.memset(tb[:], 1.0)
            tzero = spool.tile([1, 1], f32)
            nc.gpsimd.memset(tzero[:], 0.0)
            nc.sync.dma_start(t[:, :], tzero[:])

            def body():
                body_fn(nc, x, o, st, xpool, opool, ppool, xfix, obfix)
                if count:
                    nc.gpsimd.dma_start(
                        t[:, :], tb[:], accum_op=mybir.AluOpType.add
                    )

            if reps == 1:
                body()
            else:
                with tc.For_i(
                    0,
                    (reps - 1) // unroll,
                    1,
                    hint_engines=(ET.PE, ET.Activation, ET.DVE, ET.Pool, ET.SP),
                    staggered_reset=staggered_reset,
                ):
                    body()
    nc.compile()
    return nc


def build_nc_timing(reps, npc=NPC):
    def body_fn(nc, x, o, st, xpool, opool, ppool, xfix, obfix):
        _body(nc, x, o, st, xpool, opool, ppool, npc)

    return _timing_shell(npc, reps, body_fn)


def build_nc_micro(which, reps, npc=NPC):
    if which.startswith("u2"):
        which = which[2:]
        unroll = 2
    else:
        unroll = 1
    if which.startswith("sr"):
        which = which[2:]
        stag = True
    else:
        stag = False
    if which.endswith("_nc"):
        which = which[:-3]
        count = False
    else:
        count = True

    flags = {
        "mm": dict(do_load=False, do_copy=False, do_store=False),
        "mmcopy": dict(do_load=False, do_store=False),
        "load": dict(do_mm=False, do_copy=False, do_store=False),
        "store": dict(do_load=False, do_mm=False, do_copy=False),
        "nostore": dict(do_store=False),
        "mcs": dict(do_load=False),
        "lmst": dict(do_copy=False),
        "loadstore": dict(do_mm=False, do_copy=False),
        "full": dict(),
    }[which]

    def body_fn(nc, x, o, st, xpool, opool, ppool, xfix, obfix):
        for _ in range(unroll):
            _body(
                nc, x, o, st, xpool, opool, ppool, npc,
                xfix=xfix, obfix=obfix, **flags,
            )

    return _timing_shell(
        npc, reps, body_fn, staggered_reset=stag, unroll=unroll, count=count,
        fixtures=(which != "full"),
    )


_NC_CACHE = {}


def _get_nc(npc=NPC):
    if npc not in _NC_CACHE:
        _NC_CACHE[npc] = build_nc(npc)
    return _NC_CACHE[npc]


def make_in_maps(x, W):
    wb = (np.sign(W) * _channel_mask()).astype(np.float32)
    S = _build_stationary(wb).transpose(1, 0, 2).astype(_np_bf16())
    S = np.ascontiguousarray(S)  # [KP, NPASS, 128]
    shards = x.reshape(N_CORES, NPC, CI, H, WI)
    return [{"x": _pack_x(shards[i]), "s": S} for i in range(N_CORES)]


def _run(x, W, trace=False):
    from concourse.bass_utils import run_bass_kernel_spmd

    x = np.asarray(x, dtype=np.float32)
    W = np.asarray(W, dtype=np.float32)
    in_maps = make_in_maps(x, W)
    nc = _get_nc()
    res = run_bass_kernel_spmd(
        nc, in_maps, core_ids=list(range(N_CORES)), trace=trace
    )
    out = np.concatenate(
        [_unpack_o(r["o"], NPC) for r in res.results], axis=0
    )
    return out, res


def kernel(x, W):
    out, _ = _run(x, W, trace=False)
    return out


# revision 15
# speedup vs baseline: 1.8860x; 1.0849x over previous
"""Trainium2 Bass kernel for LeNet-C3 binarized 5x5 VALID conv.

out[256,16,124,124] = conv2d(x[256,6,128,128], sign(W)*mask), NCHW/OIHW.

Strategy (per core, data-parallel over batch, 8 cores x 32 images):
  Parity-split 3-pass matmul scheme, bf16 operands, f32 PSUM accumulation.

  Split w by parity: x[ci, h, 2u+c] -> partition (ci, r, c), free u.
  Per output row-block b (j=4 rows, 31 blocks), input rows 4b..4b+7:
    K = (ci, r in 8, c in 2) = 96 partitions
    M = (co 16, j 4, v 2)   = 128  -> output (co, 4b+j, 2*u0+v)
    3 PSUM-accumulated matmuls (pass p streams free offset u0+p):
      ps[(co,j,v), (n,u0)] += S_p[(ci,r,c),(co,j,v)]^T @ xb[(ci,r,c),(n,u0+p)]
    with S_p[...] = wb[co, ci, r-j, 2p+c-v] (zero where kh/kw out of range).

  vs the 5-pass f32 baseline this is 1.67x fewer PE columns (3*62 vs 5*124
  per 8-image group covering 2x w per column) and ~1.7x less HBM traffic
  (bf16 both directions; input rows 2x-replicated by the 8-row/4-stride
  blocking, outputs written exactly once).

  DMA layouts are partition-major so every transfer is contiguous per
  partition row: x dram [96, 31*2048], o dram [128, 31*1984].
"""

import sys

sys.path.insert(0, "/opt/trn_rl_repo")

import numpy as np

# ---- problem constants (hardcoded per contract) ----
N_CORES = 8
N, CI, H, WI = 256, 6, 128, 128
CO, KH, KW = 16, 5, 5
HO, WO = 124, 124
NPC = N // N_CORES  # images per core (32)

JB = 4              # output rows per block
VB = 2              # output w per psum column (parity pair)
RB = JB + KH - 1    # input rows per block (8)
KP = CI * RB * 2    # contraction partitions (96)
NBLK = HO // JB     # 31 row blocks
NU = WI // 2        # 64 u positions per image
UO = WO // 2        # 62 psum columns per image
NSUB = 8            # images per matmul tile (moving N = 8*62 = 496 <= 1024)
NGRP = NPC // NSUB  # 4 matmul groups per block
NPASS = 3
CB = 2              # row blocks per input DMA chunk (last chunk may be short)
SB = 2              # row blocks per output store
ALT_RINGS = True    # alternate both loads and stores across both HWDGE rings
STORE_GPSIMD = False  # route stores through SWDGE (third DMA path): slower
S_MAJOR = True      # stationary-major matmul ordering (fewer weight switches)

FEATURE_MAPS = [
    [0, 1, 2], [1, 2, 3], [2, 3, 4], [3, 4, 5], [0, 4, 5], [0, 1, 5],
    [0, 1, 2, 3], [1, 2, 3, 4], [2, 3, 4, 5], [0, 3, 4, 5], [0, 1, 4, 5],
    [0, 1, 2, 5], [0, 1, 3, 4], [1, 2, 4, 5], [0, 2, 3, 5],
    [0, 1, 2, 3, 4, 5],
]


def _channel_mask():
    m = np.zeros((CO, CI, 1, 1), np.float32)
    for i, maps in enumerate(FEATURE_MAPS):
        m[i, maps, 0, 0] = 1.0
    return m


def _np_bf16():
    import ml_dtypes

    return ml_dtypes.bfloat16


def _build_stationary(wb):
    """S[p, (ci,r,c), (co,j,v)] = wb[co, ci, r-j, 2p+c-v] where valid."""
    S = np.zeros((NPASS, KP, 128), np.float32)
    for p in range(NPASS):
        for ci in range(CI):
            for r in range(RB):
                for c in range(2):
                    krow = ci * 16 + r * 2 + c
                    for co in range(CO):
                        for j in range(JB):
                            kh = r - j
                            if not (0 <= kh < KH):
                                continue
                            for v in range(VB):
                                kw = 2 * p + c - v
                                if 0 <= kw < KW:
                                    S[p, krow, co * 8 + j * 2 + v] = wb[
                                        co, ci, kh, kw
                                    ]
    return S


def _pack_x(shard):
    """[npc, CI, H, WI] f32 -> [KP, NBLK*npc*NU] bf16 partition-major."""
    npc = shard.shape[0]
    xv = shard.transpose(1, 2, 0, 3)  # [ci, h, n, w]
    xb = np.empty((NBLK, CI, RB, 2, npc, NU), np.float32)
    for b in range(NBLK):
        rows = xv[:, 4 * b: 4 * b + RB]               # [ci, r, n, w]
        xb[b] = rows.reshape(CI, RB, npc, NU, 2).transpose(0, 1, 4, 2, 3)
    # -> partition (ci, r, c), free (b, n, u)
    xp = xb.transpose(1, 2, 3, 0, 4, 5).reshape(KP, NBLK * npc * NU)
    return xp.astype(_np_bf16())


def _unpack_o(o_np, npc):
    """[128, NBLK*npc*UO] bf16 -> [npc, CO, HO, WO] f32."""
    blocks = np.asarray(o_np, dtype=np.float32).reshape(
        CO, JB, VB, NBLK, npc, UO
    )
    # out[n, co, 4b+j, 2u+v]
    out = blocks.transpose(4, 0, 3, 1, 5, 2).reshape(npc, CO, HO, WO)
    return np.ascontiguousarray(out)


def _body(
    nc,
    x,
    o,
    st,
    xpool,
    opool,
    ppool,
    npc,
    do_load=True,
    do_mm=True,
    do_copy=True,
    do_store=True,
    xfix=None,
    obfix=None,
):
    import concourse.mybir as mybir

    f32 = mybir.dt.float32
    bf16 = mybir.dt.bfloat16

    chunk_starts = list(range(0, NBLK, CB))

    def issue_load(idx):
        cs = chunk_starts[idx]
        nblk_c = min(CB, NBLK - cs)
        xt = xpool.tile([KP, CB, npc, NU], bf16, tag="xt")
        leng = (nc.sync, nc.scalar)[idx % 2 if ALT_RINGS else 0]
        leng.dma_start(
            xt[:, 0:nblk_c, :, :].rearrange("p b n u -> p (b n u)"),
            x[:, cs * npc * NU: (cs + nblk_c) * npc * NU],
        )
        return xt

    PREFETCH = 2
    xts = {}
    if do_load:
        for i in range(min(PREFETCH, len(chunk_starts))):
            xts[i] = issue_load(i)

    ob = None
    n_store = 0
    for b in range(NBLK):
        ci_, bb = divmod(b, CB)
        if do_load:
            if bb == 0:
                if ci_ + PREFETCH < len(chunk_starts):
                    xts[ci_ + PREFETCH] = issue_load(ci_ + PREFETCH)
                if ci_ - 1 in xts:
                    del xts[ci_ - 1]
            xt = xts[ci_]
        else:
            xt = xfix
            bb = 0
        sb = b % SB
        if do_copy and sb == 0:
            ob = opool.tile([128, SB, npc, UO], bf16, tag="ob")
        elif not do_copy:
            ob = obfix
        if do_mm:
            if S_MAJOR:
                pss = [
                    ppool.tile([128, NSUB, UO], f32, tag="ps", name=f"ps{ng}")
                    for ng in range(NGRP)
                ]
                for p in range(NPASS):
                    for ng in range(NGRP):
                        n0 = ng * NSUB
                        nc.tensor.matmul(
                            pss[ng][:],
                            st[:, p, :],
                            xt[:, bb, n0: n0 + NSUB, p: p + UO],
                            start=(p == 0),
                            stop=(p == NPASS - 1),
                        )
                if do_copy:
                    for ng in range(NGRP):
                        n0 = ng * NSUB
                        nc.vector.tensor_copy(
                            ob[:, sb, n0: n0 + NSUB, :], pss[ng][:]
                        )
            else:
                for ng0 in range(0, NGRP, 2):
                    n0a, n0b = ng0 * NSUB, (ng0 + 1) * NSUB
                    psa = ppool.tile([128, NSUB, UO], f32, tag="ps")
                    psb = ppool.tile([128, NSUB, UO], f32, tag="ps")
                    for p in range(NPASS):
                        nc.tensor.matmul(
                            psa[:],
                            st[:, p, :],
                            xt[:, bb, n0a: n0a + NSUB, p: p + UO],
                            start=(p == 0),
                            stop=(p == NPASS - 1),
                        )
                        nc.tensor.matmul(
                            psb[:],
                            st[:, p, :],
                            xt[:, bb, n0b: n0b + NSUB, p: p + UO],
                            start=(p == 0),
                            stop=(p == NPASS - 1),
                        )
                    if do_copy:
                        nc.vector.tensor_copy(
                            ob[:, sb, n0a: n0a + NSUB, :], psa[:]
                        )
                        nc.vector.tensor_copy(
                            ob[:, sb, n0b: n0b + NSUB, :], psb[:]
                        )
        if do_store and (sb == SB - 1 or b == NBLK - 1):
            nsb = sb + 1
            b0 = b - sb
            if STORE_GPSIMD:
                seng = nc.gpsimd
            else:
                seng = (nc.scalar, nc.sync)[n_store % 2 if ALT_RINGS else 0]
            n_store += 1
            seng.dma_start(
                o[:, b0 * npc * UO: (b0 + nsb) * npc * UO],
                ob[:, 0:nsb, :, :].rearrange("p b n u -> p (b n u)"),
            )


def build_nc(npc=NPC, reps=1):
    import concourse.mybir as mybir
    import concourse.tile as tile
    from concourse import bacc

    bf16 = mybir.dt.bfloat16

    nc = bacc.Bacc(None, target_bir_lowering=False)
    x = nc.dram_tensor(
        "x", [KP, NBLK * npc * NU], bf16, kind="ExternalInput"
    )
    s = nc.dram_tensor("s", [KP, NPASS, 128], bf16, kind="ExternalInput")
    o = nc.dram_tensor(
        "o", [128, NBLK * npc * UO], bf16, kind="ExternalOutput"
    )

    with tile.TileContext(nc) as tc:
        with (
            tc.tile_pool(name="spool", bufs=1) as spool,
            tc.tile_pool(name="xpool", bufs=4) as xpool,
            tc.tile_pool(name="opool", bufs=6) as opool,
            tc.tile_pool(name="ppool", bufs=8, space="PSUM") as ppool,
        ):
            st = spool.tile([KP, NPASS, 128], bf16)
            nc.sync.dma_start(st[:], s[:, :, :].rearrange("p a m -> p (a m)"))
            for _rep in range(reps):
                _body(nc, x, o, st, xpool, opool, ppool, npc)
    nc.compile()
    return nc


def _timing_shell(
    npc, reps, body_fn, staggered_reset=False, unroll=1, count=True,
    fixtures=True,
):
    """Common For_i timing harness: internal DRAM output + rep counter."""
    import concourse.mybir as mybir
    import concourse.tile as tile
    from concourse import bacc

    f32 = mybir.dt.float32
    bf16 = mybir.dt.bfloat16
    ET = mybir.EngineType

    nc = bacc.Bacc(None, target_bir_lowering=False)
    x = nc.dram_tensor(
        "x", [KP, NBLK * npc * NU], bf16, kind="ExternalInput"
    )
    s = nc.dram_tensor("s", [KP, NPASS, 128], bf16, kind="ExternalInput")
    t = nc.dram_tensor("t", [1, 1], f32, kind="ExternalOutput")

    with tile.TileContext(nc) as tc:
        with (
            tc.tile_pool(name="spool", bufs=1) as spool,
            tc.tile_pool(name="xpool", bufs=4) as xpool,
            tc.tile_pool(name="opool", bufs=6) as opool,
            tc.tile_pool(name="ppool", bufs=8, space="PSUM") as ppool,
            tc.tile_pool(name="dpool", bufs=1, space="DRAM") as dpool,
        ):
            o = dpool.tile([128, NBLK * npc * UO], bf16)
            st = spool.tile([KP, NPASS, 128], bf16)
            nc.sync.dma_start(st[:], s[:, :, :].rearrange("p a m -> p (a m)"))
            if fixtures:
                xfix = spool.tile([KP, 1, npc, NU], bf16, tag="xfix")
                nc.sync.dma_start(
                    xfix[:].rearrange("p b n u -> p (b n u)"),
                    x[:, 0: npc * NU],
                )
                obfix = spool.tile([128, SB, npc, UO], bf16, tag="obfix")
                nc.gpsimd.memset(obfix[:], 0.25)
            else:
                xfix = obfix = None

            tb = spool.tile([1, 1], f32)
            nc.gpsimd.memset(tb[:], 1.0)
            tzero = spool.tile([1, 1], f32)
            nc.gpsimd.memset(tzero[:], 0.0)
            nc.sync.dma_start(t[:, :], tzero[:])

            def body():
                body_fn(nc, x, o, st, xpool, opool, ppool, xfix, obfix)
                if count:
                    nc.gpsimd.dma_start(
                        t[:, :], tb[:], accum_op=mybir.AluOpType.add
                    )

            if reps == 1:
                body()
            else:
                with tc.For_i(
                    0,
                    (reps - 1) // unroll,
                    1,
                    hint_engines=(ET.PE, ET.Activation, ET.DVE, ET.Pool, ET.SP),
                    staggered_reset=staggered_reset,
                ):
                    body()
    nc.compile()
    return nc


def build_nc_timing(reps, npc=NPC):
    def body_fn(nc, x, o, st, xpool, opool, ppool, xfix, obfix):
        _body(nc, x, o, st, xpool, opool, ppool, npc)

    return _timing_shell(npc, reps, body_fn)


def build_nc_micro(which, reps, npc=NPC):
    if which.startswith("u2"):
        which = which[2:]
        unroll = 2
    else:
        unroll = 1
    if which.startswith("sr"):
        which = which[2:]
        stag = True
    else:
        stag = False
    if which.endswith("_nc"):
        which = which[:-3]
        count = False
    else:
        count = True

    flags = {
        "mm": dict(do_load=False, do_copy=False, do_store=False),
        "mmcopy": dict(do_load=False, do_store=False),
        "load": dict(do_mm=False, do_copy=False, do_store=False),
        "store": dict(do_load=False, do_mm=False, do_copy=False),
        "nostore": dict(do_store=False),
        "mcs": dict(do_load=False),
        "lmst": dict(do_copy=False),
        "loadstore": dict(do_mm=False, do_copy=False),
        "full": dict(),
    }[which]

    def body_fn(nc, x, o, st, xpool, opool, ppool, xfix, obfix):
        for _ in range(unroll):
            _body(
                nc, x, o, st, xpool, opool, ppool, npc,
                xfix=xfix, obfix=obfix, **flags,
            )

    return _timing_shell(
        npc, reps, body_fn, staggered_reset=stag, unroll=unroll, count=count,
        fixtures=(which != "full"),
    )


_NC_CACHE = {}


def _get_nc(npc=NPC):
    if npc not in _NC_CACHE:
        _NC_CACHE[npc] = build_nc(npc)
    return _NC_CACHE[npc]


def make_in_maps(x, W):
    wb = (np.sign(W) * _channel_mask()).astype(np.float32)
    S = _build_stationary(wb).transpose(1, 0, 2).astype(_np_bf16())
    S = np.ascontiguousarray(S)  # [KP, NPASS, 128]
    shards = x.reshape(N_CORES, NPC, CI, H, WI)
    return [{"x": _pack_x(shards[i]), "s": S} for i in range(N_CORES)]


def _run(x, W, trace=False):
    from concourse.bass_utils import run_bass_kernel_spmd

    x = np.asarray(x, dtype=np.float32)
    W = np.asarray(W, dtype=np.float32)
    in_maps = make_in_maps(x, W)
    nc = _get_nc()
    res = run_bass_kernel_spmd(
        nc, in_maps, core_ids=list(range(N_CORES)), trace=trace
    )
    out = np.concatenate(
        [_unpack_o(r["o"], NPC) for r in res.results], axis=0
    )
    return out, res


def kernel(x, W):
    out, _ = _run(x, W, trace=False)
    return out
